# revision 1
# baseline (speedup 1.0000x reference)
"""Bass/Trainium2 kernel for nn_BatchLinearMasked (B=2048, N=64, D=256, 4 steps).

x <- x + relu(einsum('bni,nji->bnj', x, w*mask) + b*bmask), repeated 4 times.

Sharding: expert-parallel over the 64 independent groups -> 8 groups per
NeuronCore.  Each group's recurrence is fully core-local (no collectives).

Layout: all on-chip state is feature-major ([feature, batch]); the host
pre-transposes x (pure data movement, part of sharding) so the device never
transposes.  All arithmetic (mask multiplies, matmuls, bias, relu, adds)
happens on-device.

Device compute per group n (D=256 split into 2 partition blocks of 128):
  wk = wT * maskT                       (DVE, once)
  z_0 = xT                              (DMA)
  for k in 0..3:
     p[jb] = sum_ib wk[ib][:,jb]^T @ z[ib]      (PE, PSUM)
     t[jb] = relu(p[jb] + bias[jb])             (ACT, per-partition bias)
     z[ib] += t[jb=ib]                          (DVE)
  yT = z_4                              (DMA)
"""

import numpy as np

B = 2048          # batch
N = 64            # n_linears (groups)
D = 256           # feature dim
NCORES = 8
NG = N // NCORES  # groups per core = 8
NITER = 4         # recurrence steps
FCHUNK = 512      # fp32 matmul moving free-dim max
PAIR = 2          # groups interleaved to hide elementwise latency behind PE

_nc_cache = {}


def _build_nc(reps=1):
    """Build + compile the per-core Bass program (SPMD, identical on all cores)."""
    import concourse.tile as tile
    from concourse import bacc, mybir

    f32 = mybir.dt.float32
    nc = bacc.Bacc("TRN2", target_bir_lowering=False, debug=False, num_devices=NCORES)

    XT = nc.dram_tensor("xt", [NG, D, B], f32, kind="ExternalInput")
    WT = nc.dram_tensor("wt", [NG, D, D], f32, kind="ExternalInput")
    WMT = nc.dram_tensor("wmt", [NG, D, D], f32, kind="ExternalInput")
    BB = nc.dram_tensor("bb", [128, 2 * NG], f32, kind="ExternalInput")
    BBM = nc.dram_tensor("bbm", [128, 2 * NG], f32, kind="ExternalInput")
    YT = nc.dram_tensor("yt", [NG, D, B], f32, kind="ExternalOutput")

    RELU = mybir.ActivationFunctionType.Relu

    with tile.TileContext(nc) as tc:
        with (
            tc.tile_pool(name="bias", bufs=1) as bias_pool,
            tc.tile_pool(name="wraw", bufs=4) as wraw_pool,
            tc.tile_pool(name="wk", bufs=2 * PAIR + 2) as wk_pool,
            tc.tile_pool(name="z", bufs=4 * PAIR + 2) as z_pool,
            tc.tile_pool(name="t", bufs=4) as t_pool,
            tc.tile_pool(name="ps", bufs=2, space="PSUM") as ps_pool,
        ):
            bb_t = bias_pool.tile([128, 2 * NG], f32, tag="bb")
            nc.sync.dma_start(bb_t[:], BB[:])
            bbm_t = bias_pool.tile([128, 2 * NG], f32, tag="bbm")
            nc.sync.dma_start(bbm_t[:], BBM[:])
            bvec = bias_pool.tile([128, 2 * NG], f32, tag="bvec")
            nc.vector.tensor_mul(bvec[:], bb_t[:], bbm_t[:])

            for _rep in range(reps):
                for pair in range(NG // PAIR):
                    groups = [PAIR * pair + i for i in range(PAIR)]
                    state = {}
                    for g in groups:
                        wk = []
                        for ib in range(2):
                            wr = wraw_pool.tile([128, D], f32, tag="wraw")
                            nc.sync.dma_start(wr[:], WT[g, ib * 128:(ib + 1) * 128, :])
                            mr = wraw_pool.tile([128, D], f32, tag="mraw")
                            nc.sync.dma_start(mr[:], WMT[g, ib * 128:(ib + 1) * 128, :])
                            wm = wk_pool.tile([128, D], f32, tag="wk")
                            nc.vector.tensor_mul(wm[:], wr[:], mr[:])
                            wk.append(wm)
                        zs = []
                        for ib in range(2):
                            z = z_pool.tile([128, B], f32, tag="z")
                            nc.sync.dma_start(z[:], XT[g, ib * 128:(ib + 1) * 128, :])
                            zs.append(z)
                        state[g] = (wk, zs)

                    for k in range(NITER):
                        for g in groups:
                            wk, zs = state[g]
                            ts = []
                            for jb in range(2):
                                p = ps_pool.tile([128, B], f32, tag="p")
                                for ib in range(2):
                                    lhsT = wk[ib][:, jb * 128:(jb + 1) * 128]
                                    for c in range(B // FCHUNK):
                                        nc.tensor.matmul(
                                            p[:, c * FCHUNK:(c + 1) * FCHUNK],
                                            lhsT,
                                            zs[ib][:, c * FCHUNK:(c + 1) * FCHUNK],
                                            start=(ib == 0),
                                            stop=(ib == 1),
                                        )
                                tt = t_pool.tile([128, B], f32, tag="t")
                                col = g * 2 + jb
                                nc.scalar.activation(
                                    tt[:], p[:], RELU, bias=bvec[:, col:col + 1]
                                )
                                ts.append(tt)
                            nzs = []
                            for ib in range(2):
                                zn = z_pool.tile([128, B], f32, tag="z")
                                nc.vector.tensor_add(zn[:], zs[ib][:], ts[ib][:])
                                nzs.append(zn)
                            state[g] = (wk, nzs)

                    for g in groups:
                        _, zs = state[g]
                        for ib in range(2):
                            nc.sync.dma_start(
                                YT[g, ib * 128:(ib + 1) * 128, :], zs[ib][:]
                            )

    nc.compile()
    return nc


def get_nc(reps=1):
    if reps not in _nc_cache:
        _nc_cache[reps] = _build_nc(reps)
    return _nc_cache[reps]


def make_in_maps(x, weights, biases, weight_mask, bias_mask):
    """Host-side sharding/layout prep (pure data movement)."""
    xt = np.ascontiguousarray(x.transpose(1, 2, 0))            # [N, D, B]
    wt = np.ascontiguousarray(weights.transpose(0, 2, 1))      # [N, D, D] (lhsT)
    wmt = np.ascontiguousarray(weight_mask.transpose(0, 2, 1))
    # bb[p, n*2+jb] = biases[n, jb*128+p]
    bb = np.ascontiguousarray(
        biases.reshape(N, 2, 128).transpose(2, 0, 1).reshape(128, 2 * N))
    bbm = np.ascontiguousarray(
        bias_mask.reshape(N, 2, 128).transpose(2, 0, 1).reshape(128, 2 * N))
    in_maps = []
    for c in range(NCORES):
        in_maps.append({
            "xt": xt[c * NG:(c + 1) * NG],
            "wt": wt[c * NG:(c + 1) * NG],
            "wmt": wmt[c * NG:(c + 1) * NG],
            "bb": np.ascontiguousarray(bb[:, c * 2 * NG:(c + 1) * 2 * NG]),
            "bbm": np.ascontiguousarray(bbm[:, c * 2 * NG:(c + 1) * 2 * NG]),
        })
    return in_maps


def unshard(results):
    """[per-core {'yt': [NG, D, B]}] -> full [B, N, D] output."""
    yt = np.concatenate([results[c]["yt"] for c in range(NCORES)], axis=0)  # [N, D, B]
    return np.ascontiguousarray(yt.transpose(2, 0, 1))


def kernel(x, weights, biases, weight_mask, bias_mask):
    from concourse.bass_utils import run_bass_kernel_spmd

    x = np.asarray(x, dtype=np.float32)
    weights = np.asarray(weights, dtype=np.float32)
    biases = np.asarray(biases, dtype=np.float32)
    weight_mask = np.asarray(weight_mask, dtype=np.float32)
    bias_mask = np.asarray(bias_mask, dtype=np.float32)

    in_maps = make_in_maps(x, weights, biases, weight_mask, bias_mask)
    nc = get_nc(reps=1)
    res = run_bass_kernel_spmd(nc, in_maps, list(range(NCORES)))
    return unshard(res.results)


# revision 13
# speedup vs baseline: 21.3612x; 21.3612x over previous
"""Bass/Trainium2 kernel for nn_BatchLinearMasked (B=2048, N=64, D=256, 4 steps).

x <- x + relu(einsum('bni,nji->bnj', x, w*mask) + b*bmask), repeated 4 times.

Sharding: expert-parallel over the 64 independent groups -> 8 groups per
NeuronCore.  Each group's recurrence is fully core-local (no collectives).

Layout: all on-chip state is feature-major ([feature, batch]); the host
pre-transposes x (pure data movement, part of sharding) so the device never
transposes.  All arithmetic (mask multiplies, matmuls, bias, relu, adds)
happens on-device.

Device compute per group n (D=256 split into 2 partition blocks of 128):
  wk = wT * maskT                       (DVE, once)
  z_0 = xT                              (DMA)
  for k in 0..3:
     p[jb] = sum_ib wk[ib][:,jb]^T @ z[ib]      (PE, PSUM)
     t[jb] = relu(p[jb] + bias[jb])             (ACT, per-partition bias)
     z[ib] += t[jb=ib]                          (DVE)
  yT = z_4                              (DMA)
"""

import numpy as np

B = 2048          # batch
N = 64            # n_linears (groups)
D = 256           # feature dim
NCORES = 8
NG = N // NCORES  # groups per core = 8
NITER = 4         # recurrence steps
FCHUNK = 512      # fp32 matmul moving free-dim max
PAIR = 2          # groups interleaved to hide elementwise latency behind PE

_nc_cache = {}


def _build_nc(reps=1):
    """Build + compile the per-core Bass program (SPMD, identical on all cores)."""
    import concourse.tile as tile
    from concourse import bacc, mybir

    f32 = mybir.dt.float32
    f32r = mybir.dt.float32r
    nc = bacc.Bacc("TRN2", target_bir_lowering=False, debug=False, num_devices=NCORES)

    XT = nc.dram_tensor("xt", [NG, D, B], f32r, kind="ExternalInput")
    WT = nc.dram_tensor("wt", [NG, D, D], f32, kind="ExternalInput")
    WMT = nc.dram_tensor("wmt", [NG, D, D], f32, kind="ExternalInput")
    BB = nc.dram_tensor("bb", [128, 2 * NG], f32, kind="ExternalInput")
    BBM = nc.dram_tensor("bbm", [128, 2 * NG], f32, kind="ExternalInput")
    YT = nc.dram_tensor("yt", [NG, D, B], f32r, kind="ExternalOutput")

    RELU = mybir.ActivationFunctionType.Relu

    with tile.TileContext(nc) as tc:
        with (
            tc.tile_pool(name="bias", bufs=1) as bias_pool,
            tc.tile_pool(name="wraw", bufs=8) as wraw_pool,
            tc.tile_pool(name="wk", bufs=4 * PAIR) as wk_pool,
            tc.tile_pool(name="z", bufs=4 * PAIR + 4) as z_pool,
            tc.tile_pool(name="t", bufs=5) as t_pool,
            tc.tile_pool(name="ps", bufs=2, space="PSUM") as ps_pool,
        ):
            bb_t = bias_pool.tile([128, 2 * NG], f32, tag="bb")
            nc.sync.dma_start(bb_t[:], BB[:])
            bbm_t = bias_pool.tile([128, 2 * NG], f32, tag="bbm")
            nc.sync.dma_start(bbm_t[:], BBM[:])
            bvec = bias_pool.tile([128, 2 * NG], f32, tag="bvec")
            nc.gpsimd.tensor_mul(bvec[:], bb_t[:], bbm_t[:])
            add_ctr = 0

            for _rep in range(reps):
                g0 = 0
                while g0 < NG:
                    groups = list(range(g0, min(g0 + PAIR, NG)))
                    g0 += PAIR
                    state = {}
                    for g in groups:
                        wk = []
                        for ib in range(2):
                            wr = wraw_pool.tile([128, D], f32, tag="wraw")
                            nc.sync.dma_start(wr[:], WT[g, ib * 128:(ib + 1) * 128, :])
                            mr = wraw_pool.tile([128, D], f32, tag="mraw")
                            nc.sync.dma_start(mr[:], WMT[g, ib * 128:(ib + 1) * 128, :])
                            wm = wk_pool.tile([128, D], f32r, tag="wk")
                            nc.gpsimd.tensor_mul(wm[:], wr[:], mr[:])
                            wk.append(wm)
                        zs = []
                        for ib in range(2):
                            z = z_pool.tile([128, B], f32r, tag="z")
                            nc.sync.dma_start(z[:], XT[g, ib * 128:(ib + 1) * 128, :])
                            zs.append(z)
                        state[g] = (wk, zs)

                    for k in range(NITER):
                        for g in groups:
                            wk, zs = state[g]
                            ts = []
                            for jb in range(2):
                                p = ps_pool.tile([128, B], f32, tag="p")
                                for ib in range(2):
                                    lhsT = wk[ib][:, jb * 128:(jb + 1) * 128]
                                    for c in range(B // FCHUNK):
                                        nc.tensor.matmul(
                                            p[:, c * FCHUNK:(c + 1) * FCHUNK],
                                            lhsT,
                                            zs[ib][:, c * FCHUNK:(c + 1) * FCHUNK],
                                            start=(ib == 0),
                                            stop=(ib == 1),
                                        )
                                tt = t_pool.tile([128, B], f32r, tag="t")
                                col = g * 2 + jb
                                nc.scalar.activation(
                                    tt[:], p[:], RELU, bias=bvec[:, col:col + 1]
                                )
                                ts.append(tt)
                            nzs = []
                            for ib in range(2):
                                zn = z_pool.tile([128, B], f32r, tag="z")
                                eng = nc.gpsimd if add_ctr % 3 == 2 else nc.vector
                                eng.tensor_add(zn[:], zs[ib][:], ts[ib][:])
                                add_ctr += 1
                                nzs.append(zn)
                            state[g] = (wk, nzs)

                    for g in groups:
                        _, zs = state[g]
                        for ib in range(2):
                            # stores go out on the ACT HWDGE queue so their
                            # waits don't head-of-line-block the next pair's
                            # loads in SP's in-order stream
                            nc.scalar.dma_start(
                                YT[g, ib * 128:(ib + 1) * 128, :], zs[ib][:]
                            )

    nc.compile()
    return nc


def get_nc(reps=1):
    if reps not in _nc_cache:
        _nc_cache[reps] = _build_nc(reps)
    return _nc_cache[reps]


def make_in_maps(x, weights, biases, weight_mask, bias_mask):
    """Host-side sharding/layout prep (pure data movement)."""
    xt = np.ascontiguousarray(x.transpose(1, 2, 0))            # [N, D, B]
    wt = np.ascontiguousarray(weights.transpose(0, 2, 1))      # [N, D, D] (lhsT)
    wmt = np.ascontiguousarray(weight_mask.transpose(0, 2, 1))
    # bb[p, n*2+jb] = biases[n, jb*128+p]
    bb = np.ascontiguousarray(
        biases.reshape(N, 2, 128).transpose(2, 0, 1).reshape(128, 2 * N))
    bbm = np.ascontiguousarray(
        bias_mask.reshape(N, 2, 128).transpose(2, 0, 1).reshape(128, 2 * N))
    in_maps = []
    for c in range(NCORES):
        in_maps.append({
            "xt": xt[c * NG:(c + 1) * NG],
            "wt": wt[c * NG:(c + 1) * NG],
            "wmt": wmt[c * NG:(c + 1) * NG],
            "bb": np.ascontiguousarray(bb[:, c * 2 * NG:(c + 1) * 2 * NG]),
            "bbm": np.ascontiguousarray(bbm[:, c * 2 * NG:(c + 1) * 2 * NG]),
        })
    return in_maps


def unshard(results):
    """[per-core {'yt': [NG, D, B]}] -> full [B, N, D] output."""
    yt = np.concatenate([results[c]["yt"] for c in range(NCORES)], axis=0)  # [N, D, B]
    return np.ascontiguousarray(yt.transpose(2, 0, 1))


def kernel(x, weights, biases, weight_mask, bias_mask):
    from concourse.bass_utils import run_bass_kernel_spmd

    x = np.asarray(x, dtype=np.float32)
    weights = np.asarray(weights, dtype=np.float32)
    biases = np.asarray(biases, dtype=np.float32)
    weight_mask = np.asarray(weight_mask, dtype=np.float32)
    bias_mask = np.asarray(bias_mask, dtype=np.float32)

    in_maps = make_in_maps(x, weights, biases, weight_mask, bias_mask)
    nc = get_nc(reps=1)
    res = run_bass_kernel_spmd(nc, in_maps, list(range(NCORES)))
    return unshard(res.results)


# revision 32
# speedup vs baseline: 362.3131x; 16.9613x over previous
"""Bass/Trainium2 kernel for nn_BatchLinearMasked (B=2048, N=64, D=256, 4 steps).

x <- x + relu(einsum('bni,nji->bnj', x, w*mask) + b*bmask), repeated 4 times.

Sharding: expert-parallel over the 64 independent groups -> 8 groups per
NeuronCore.  Each group's recurrence is fully core-local (no collectives).

Layout: all on-chip state is feature-major ([feature, batch]); the host
pre-transposes x (pure data movement, part of sharding) so the device never
transposes.  All arithmetic (mask multiplies, matmuls, bias, relu, adds)
happens on-device.  Matmuls run in float32r (full-rate PE path).

Bias-shift trick: track z_k = x_k - f_k where f_k is a per-partition constant
vector per group (f_0 = 0, f_{k+1} = f_k + g_k + b, g_k = W f_k).  Then

    z_{k+1} = z_k + max(p_k, s_k),   p_k = W z_k,  s_k = -(g_k + b)

which is ONE fused scalar_tensor_tensor op per tile (vs relu pass + add
pass), with g_k computed by tiny N=1 matmuls accumulated alongside the main
MMs.  Final iteration computes x_4 = (z_3 + f_3) + relu(p_3 + g_3 + b)
directly via ACT relu + one fused stt, so no extra un-shift pass is needed.

Work is split per tile between DVE (fused stt) and ACT relu + Pool add
(unfused) to balance the three elementwise-capable engines.
"""

import numpy as np

B = 2048          # batch
N = 64            # n_linears (groups)
D = 256           # feature dim
NCORES = 8
NG = N // NCORES  # groups per core = 8
NITER = 4         # recurrence steps
FCHUNK = 512      # fp32 matmul moving free-dim max
PAIR = 2          # groups interleaved to hide elementwise latency behind PE
PB = 1024         # psum tile batch columns (2 banks)

# engine split for the state update, chosen per (group, iteration): out of
# every 12 group-iterations, this many take the fused DVE path; the rest take
# ACT relu + Pool add (leaving the +sb shift in z, with f frozen).
FUSED_OF_12 = 8

_nc_cache = {}


def _build_nc(reps=1):
    """Build + compile the per-core Bass program (SPMD, identical on all cores)."""
    import concourse.tile as tile
    from concourse import bacc, mybir

    f32 = mybir.dt.float32
    f32r = mybir.dt.float32r
    AL = mybir.AluOpType
    RELU = mybir.ActivationFunctionType.Relu
    nc = bacc.Bacc("TRN2", target_bir_lowering=False, debug=False, num_devices=NCORES)

    XT = nc.dram_tensor("xt", [NG, D, B], f32r, kind="ExternalInput")
    WT = nc.dram_tensor("wt", [NG, D, D], f32, kind="ExternalInput")
    WMT = nc.dram_tensor("wmt", [NG, D, D], f32, kind="ExternalInput")
    BB = nc.dram_tensor("bb", [128, 4 * NG], f32, kind="ExternalInput")
    BBM = nc.dram_tensor("bbm", [128, 4 * NG], f32, kind="ExternalInput")
    YT = nc.dram_tensor("yt", [NG, D, B], f32r, kind="ExternalOutput")

    NH = B // PB  # psum tiles per batch

    with tile.TileContext(nc) as tc:
        with (
            tc.tile_pool(name="bias", bufs=1) as bias_pool,
            tc.tile_pool(name="wraw", bufs=8) as wraw_pool,
            tc.tile_pool(name="wk", bufs=8) as wk_pool,
            tc.tile_pool(name="z", bufs=4 * PAIR + 4) as z_pool,
            tc.tile_pool(name="t", bufs=6) as t_pool,
            tc.tile_pool(name="sm", bufs=4 * PAIR + 8) as sm_pool,
            tc.tile_pool(name="ps", bufs=3, space="PSUM") as ps_pool,
            tc.tile_pool(name="fp", bufs=2, space="PSUM") as fp_pool,
        ):
            upd_ctr = 0  # fused-vs-unfused round robin

            # masked weights are prepared per pair and kept resident
            wk_all = {}

            def prep_weights(g):
                if g in wk_all:
                    return
                wk = []
                for ib in range(2):
                    wr = wraw_pool.tile([128, D], f32, tag="wraw", name="wr")
                    nc.sync.dma_start(wr[:], WT[g, ib * 128:(ib + 1) * 128, :])
                    mr = wraw_pool.tile([128, D], f32, tag="mraw", name="mr")
                    nc.sync.dma_start(mr[:], WMT[g, ib * 128:(ib + 1) * 128, :])
                    wm = wk_pool.tile([128, D], f32r, tag="wk", name="wm")
                    nc.gpsimd.tensor_mul(wm[:], wr[:], mr[:])
                    wk.append(wm)
                wk_all[g] = wk

            bb_t = bias_pool.tile([128, 4 * NG], f32, tag="bb")
            nc.sync.dma_start(bb_t[:], BB[:])
            bbm_t = bias_pool.tile([128, 4 * NG], f32, tag="bbm")
            nc.sync.dma_start(bbm_t[:], BBM[:])
            bvec = bias_pool.tile([128, 4 * NG], f32r, tag="bvec")
            nc.gpsimd.tensor_mul(bvec[:], bb_t[:], bbm_t[:])
            s0_all = bias_pool.tile([128, 4 * NG], f32r, tag="s0")
            nc.vector.tensor_scalar(s0_all[:], bvec[:], -1.0, None, AL.mult)

            for _rep in range(reps):
                wk_all.clear()  # each rep re-loads weights like a fresh run
                for g0 in range(0, NG, PAIR):
                    groups = list(range(g0, min(g0 + PAIR, NG)))
                    state = {}
                    for g in groups:
                        prep_weights(g)
                        zs = []
                        for ib in range(2):
                            z = z_pool.tile([128, B], f32r, tag="z")
                            nc.sync.dma_start(
                                z[:], XT[g, ib * 128:(ib + 1) * 128, :])
                            zs.append(z)
                        state[g] = (wk_all[g], zs, [None, None])  # f_0 = 0

                    for k in range(NITER):
                        last = k == NITER - 1
                        for g in groups:
                            # fk: per-j-block shift vectors [f(block0), f(block1)],
                            # each a [128, 2] dup-column AP or None (= zero).
                            # Fused (DVE stt) vs unfused (ACT relu + Pool add)
                            # is chosen per (group, iter, jb).  The unfused
                            # update adds relu(p+sb) = max(p,s) + sb, leaving z
                            # ahead by sb; that is absorbed by NOT advancing
                            # f[jb] for this step.
                            wk, zs, fk = state[g]
                            gcols = slice(4 * g, 4 * g + 4)
                            fused = [upd_ctr % 12 < FUSED_OF_12,
                                     (upd_ctr + 6) % 12 < FUSED_OF_12]
                            upd_ctr += 1
                            have_f = [f is not None for f in fk]

                            # --- matmuls ---
                            fp = None
                            if any(have_f):
                                fp = fp_pool.tile([128, 4], f32, tag="fp")
                                nzib = [ib for ib in range(2) if have_f[ib]]
                            ps = []
                            for jb in range(2):
                                ph = [ps_pool.tile([128, PB], f32, tag="p", name="p")
                                      for _ in range(NH)]
                                for ib in range(2):
                                    lhsT = wk[ib][:, jb * 128:(jb + 1) * 128]
                                    if fp is not None and have_f[ib]:
                                        # g_k[jb] += wk[ib][:,jb]^T f_k[ib]
                                        # (issued first so s/sb are off the
                                        # update's critical path)
                                        nc.tensor.matmul(
                                            fp[:, 2 * jb:2 * jb + 2],
                                            lhsT,
                                            fk[ib],
                                            start=(ib == nzib[0]),
                                            stop=(ib == nzib[-1]),
                                        )
                                    for h in range(NH):
                                        for c in range(PB // FCHUNK):
                                            c0 = c * FCHUNK
                                            nc.tensor.matmul(
                                                ph[h][:, c0:c0 + FCHUNK],
                                                lhsT,
                                                zs[ib][:, h * PB + c0:
                                                       h * PB + c0 + FCHUNK],
                                                start=(ib == 0),
                                                stop=(ib == 1),
                                            )
                                ps.append(ph)

                            # --- per-iteration constants ---
                            # sb = g_k + b;  s = -sb;  g_k = 0 when f = 0
                            if fp is None:
                                sb = bvec[:, gcols]
                                s = s0_all[:, gcols]
                            else:
                                sbt = sm_pool.tile([128, 4], f32r, tag="sb")
                                nc.vector.tensor_add(sbt[:], fp[:], bvec[:, gcols])
                                sb = sbt[:]
                                if any(fused) and not last:
                                    st = sm_pool.tile([128, 4], f32r, tag="s")
                                    nc.scalar.mul(st[:], sbt[:], -1.0)
                                    s = st[:]

                            # --- next-step shifts (f_{k+1}[jb] = f_k[jb] + sb[jb]
                            # if fused, else frozen) ---
                            fk_next = list(fk)
                            if not last:
                                for jb in range(2):
                                    if not fused[jb]:
                                        continue
                                    sbp = sb[:, 2 * jb:2 * jb + 2]
                                    if fk[jb] is None:
                                        fk_next[jb] = sbp
                                    else:
                                        fn = sm_pool.tile([128, 2], f32r, tag="f")
                                        nc.vector.tensor_add(fn[:], fk[jb], sbp)
                                        fk_next[jb] = fn[:]

                            # --- state update ---
                            nzs = [z_pool.tile([128, B], f32r, tag="z", name="zn")
                                   for _ in range(2)]
                            for jb in range(2):
                                sbc = sb[:, 2 * jb:2 * jb + 1]
                                for h in range(NH):
                                    hsl = slice(h * PB, (h + 1) * PB)
                                    p = ps[jb][h]
                                    zo = zs[jb][:, hsl]
                                    zn = nzs[jb][:, hsl]
                                    if last:
                                        # x4 = (z3 + f3) + relu(p + sb3)
                                        tt = t_pool.tile([128, PB], f32r, tag="t")
                                        nc.scalar.activation(
                                            tt[:], p[:], RELU, bias=sbc)
                                        if fk[jb] is None:
                                            eng = (nc.vector if fused[jb]
                                                   else nc.gpsimd)
                                            eng.tensor_add(zn, zo, tt[:])
                                        elif fused[jb]:
                                            nc.vector.scalar_tensor_tensor(
                                                zn, zo,
                                                fk[jb][:, 0:1], tt[:],
                                                AL.add, AL.add)
                                        else:
                                            tm = t_pool.tile([128, PB], f32r,
                                                             tag="tm")
                                            nc.gpsimd.tensor_add(tm[:], zo, tt[:])
                                            # cheap 2x-mode single-src add
                                            nc.vector.tensor_scalar(
                                                zn, tm[:],
                                                fk[jb][:, 0:1].bitcast(f32),
                                                None, AL.add)
                                    elif fused[jb]:
                                        # z_{k+1} = max(p, s) + z  (DVE)
                                        nc.vector.scalar_tensor_tensor(
                                            zn, p[:], s[:, 2 * jb:2 * jb + 1],
                                            zo, AL.max, AL.add)
                                    else:
                                        # z' = z + relu(p + sb)  (ACT + Pool;
                                        # the extra +sb stays in z, f frozen)
                                        tt = t_pool.tile([128, PB], f32r, tag="t")
                                        nc.scalar.activation(
                                            tt[:], p[:], RELU, bias=sbc)
                                        nc.gpsimd.tensor_add(zn, zo, tt[:])
                            state[g] = (wk, nzs, fk_next)

                    for g in groups:
                        _, zs, _ = state[g]
                        for ib in range(2):
                            # stores on the ACT HWDGE queue: their waits must
                            # not head-of-line-block the loads on SP's queue
                            nc.scalar.dma_start(
                                YT[g, ib * 128:(ib + 1) * 128, :], zs[ib][:]
                            )

    nc.compile()
    return nc


def get_nc(reps=1):
    if reps not in _nc_cache:
        _nc_cache[reps] = _build_nc(reps)
    return _nc_cache[reps]


def make_in_maps(x, weights, biases, weight_mask, bias_mask):
    """Host-side sharding/layout prep (pure data movement)."""
    xt = np.ascontiguousarray(x.transpose(1, 2, 0))            # [N, D, B]
    wt = np.ascontiguousarray(weights.transpose(0, 2, 1))      # [N, D, D] (lhsT)
    wmt = np.ascontiguousarray(weight_mask.transpose(0, 2, 1))
    # bb[p, 4n+2jb+r] = biases[n, jb*128+p] for r in {0,1} (dup-4 layout so
    # the tiny g = W f matmuls can run at the fp32r minimum free dim of 2)
    bb = np.ascontiguousarray(np.repeat(
        biases.reshape(N, 2, 128).transpose(2, 0, 1), 2, axis=2).reshape(128, 4 * N))
    bbm = np.ascontiguousarray(np.repeat(
        bias_mask.reshape(N, 2, 128).transpose(2, 0, 1), 2, axis=2).reshape(128, 4 * N))
    in_maps = []
    for c in range(NCORES):
        in_maps.append({
            "xt": xt[c * NG:(c + 1) * NG],
            "wt": wt[c * NG:(c + 1) * NG],
            "wmt": wmt[c * NG:(c + 1) * NG],
            "bb": np.ascontiguousarray(bb[:, c * 4 * NG:(c + 1) * 4 * NG]),
            "bbm": np.ascontiguousarray(bbm[:, c * 4 * NG:(c + 1) * 4 * NG]),
        })
    return in_maps


def unshard(results):
    """[per-core {'yt': [NG, D, B]}] -> full [B, N, D] output."""
    yt = np.concatenate([results[c]["yt"] for c in range(NCORES)], axis=0)  # [N, D, B]
    return np.ascontiguousarray(yt.transpose(2, 0, 1))


def kernel(x, weights, biases, weight_mask, bias_mask):
    from concourse.bass_utils import run_bass_kernel_spmd

    x = np.asarray(x, dtype=np.float32)
    weights = np.asarray(weights, dtype=np.float32)
    biases = np.asarray(biases, dtype=np.float32)
    weight_mask = np.asarray(weight_mask, dtype=np.float32)
    bias_mask = np.asarray(bias_mask, dtype=np.float32)

    in_maps = make_in_maps(x, weights, biases, weight_mask, bias_mask)
    nc = get_nc(reps=1)
    res = run_bass_kernel_spmd(nc, in_maps, list(range(NCORES)))
    return unshard(res.results)
